# revision 9
# baseline (speedup 1.0000x reference)
"""Trainium2 Bass kernel for nn_DeformAtten1D (deformable 1D attention).

Self-contained: takes FULL unsharded inputs as numpy arrays, shards
batch-parallel across 8 NeuronCores, runs a Bass/Tile kernel per core, and
reassembles the full [B, L, C] float32 output.

v2 redesign vs v1: all DMA is contiguous-run friendly (no rearrange
transposes, no scattered stores, no DRAM roundtrips, no indirect gather).
Grid-sample is computed as a banded matmul against an on-chip-built "tent"
selection matrix S[l, l'] = relu(1 - |pos(l') - l|), exploiting that
|pos - l'| <= ~9. Channel-major <-> seq-major layout changes happen on the
PE via transposes.
"""
import sys
for _p in ('/opt/trn_rl_repo', '/root/.axon_site/_ro/trn_rl_repo'):
    if _p not in sys.path:
        sys.path.insert(0, _p)

import numpy as np
import ml_dtypes

import concourse.bass as bass
import concourse.bacc as bacc
import concourse.mybir as mybir
import concourse.tile as tile
from concourse.masks import make_identity

F32 = mybir.dt.float32
BF16 = mybir.dt.bfloat16
AF = mybir.ActivationFunctionType
OP = mybir.AluOpType
BF = ml_dtypes.bfloat16


class Cfg:
    def __init__(self, B_SH, L, C, H, G, K=5):
        self.B_SH, self.L, self.C, self.H, self.G, self.K = B_SH, L, C, H, G, K
        self.GC = C // G           # 256
        self.DH = C // H           # 64
        self.PAD = K // 2
        self.L4 = L + 2 * self.PAD
        self.sn = L / (self.L4 - 1)
        self.KT = C // 128         # 8
        self.NS = L // 128         # 16
        self.SLG = self.GC // 128  # 2
        self.NB = L // 512         # 4
        self.NO = C // 512         # 2
        self.MH = self.KT // self.NO  # 4
        self.P2 = H // 2           # 8
        assert self.DH == 64 and self.P2 == self.KT
        assert L % 512 == 0 and C % 512 == 0 and self.GC % 128 == 0


def declare(nc, cfg):
    c = cfg
    t = {}
    t['xg'] = nc.dram_tensor("xg", [c.B_SH * c.L, c.C], BF16, kind="ExternalInput")
    t['xgb'] = nc.dram_tensor("xgb", [128, c.B_SH * c.NS * c.C], BF16, kind="ExternalInput")
    t['rtab'] = nc.dram_tensor("rtab", [128, c.KT * c.L], BF16, kind="ExternalInput")
    t['wc'] = nc.dram_tensor("wc", [128, c.SLG * c.K], BF16, kind="ExternalInput")
    t['offc'] = nc.dram_tensor("offc", [128, 1], F32, kind="ExternalInput")
    for nm in ('wq', 'wk', 'wv', 'wo'):
        t[nm] = nc.dram_tensor(nm, [128, c.KT * c.C], BF16, kind="ExternalInput")
    t['bq_col'] = nc.dram_tensor("bq_col", [128, c.KT], F32, kind="ExternalInput")
    t['bk_row'] = nc.dram_tensor("bk_row", [1, c.C], BF16, kind="ExternalInput")
    t['bout_row'] = nc.dram_tensor("bout_row", [1, c.C], BF16, kind="ExternalInput")
    t['boff2c'] = nc.dram_tensor("boff2c", [1, 1], F32, kind="ExternalInput")
    t['posg'] = nc.dram_tensor("posg", [128, c.NS], F32, kind="ExternalInput")
    t['iota'] = nc.dram_tensor("iota", [1, 208], BF16, kind="ExternalInput")
    t['out2d'] = nc.dram_tensor("out2d", [128, c.B_SH * c.NS * c.C], BF16, kind="ExternalOutput")
    return t


def build(tc, t, cfg, ctx):
    c = cfg
    nc = tc.nc
    L, C, KT, NS, NB, NO, GC, SLG, G, K, MH = (c.L, c.C, c.KT, c.NS, c.NB, c.NO,
                                               c.GC, c.SLG, c.G, c.K, c.MH)
    scale = C ** -0.5

    konst = ctx.enter_context(tc.tile_pool(name="konst", bufs=1))
    big = ctx.enter_context(tc.tile_pool(name="big", bufs=1))
    wp = ctx.enter_context(tc.tile_pool(name="wp", bufs=2))
    sm = ctx.enter_context(tc.tile_pool(name="sm", bufs=2))
    smb = ctx.enter_context(tc.tile_pool(name="smb", bufs=1))
    smb2 = ctx.enter_context(tc.tile_pool(name="smb2", bufs=2))
    rp = ctx.enter_context(tc.tile_pool(name="rp", bufs=2))
    yp = ctx.enter_context(tc.tile_pool(name="yp", bufs=2))
    ap_ = ctx.enter_context(tc.tile_pool(name="ap", bufs=1))
    psmm = ctx.enter_context(tc.tile_pool(name="psmm", bufs=3, space="PSUM"))
    pstb = ctx.enter_context(tc.tile_pool(name="pstb", bufs=2, space="PSUM"))
    ps1f = ctx.enter_context(tc.tile_pool(name="ps1f", bufs=2, space="PSUM"))
    psx = ctx.enter_context(tc.tile_pool(name="psx", bufs=1, space="PSUM"))

    # ---- constants ----
    wc_sb = konst.tile([128, SLG, K], BF16, tag="wc")
    nc.sync.dma_start(out=wc_sb[:], in_=t['wc'].ap())
    offc_sb = konst.tile([128, 1], F32, tag="offc")
    nc.sync.dma_start(out=offc_sb[:], in_=t['offc'].ap())
    posg_sb = konst.tile([128, NS], F32, tag="posg")
    nc.sync.dma_start(out=posg_sb[:], in_=t['posg'].ap())
    iota_sb = konst.tile([128, 208], BF16, tag="iota")
    nc.sync.dma_start(out=iota_sb[:], in_=bass.AP(
        tensor=t['iota'].ap().tensor, offset=0, ap=[[0, 128], [1, 208]]))
    bq_col_sb = konst.tile([128, KT], F32, tag="bqc")
    nc.sync.dma_start(out=bq_col_sb[:], in_=t['bq_col'].ap())
    boff2_sb = konst.tile([128, 1], F32, tag="bo2")
    nc.sync.dma_start(out=boff2_sb[:], in_=bass.AP(
        tensor=t['boff2c'].ap().tensor, offset=0, ap=[[0, 128], [1, 1]]))
    rtab_sb = konst.tile([128, KT * L], BF16, tag="rtab")
    identb = konst.tile([128, 128], BF16, tag="identb")
    make_identity(nc, identb[:])

    def load_weight(wname):
        # one DMA per weight: host layout [p, hi, kt, j] is fully contiguous
        # per partition (16KB runs, 128 descriptors for 2MB)
        wh = wp.tile([128, NO, KT, 512], BF16, tag="wblk")
        nc.sync.dma_start(out=wh[:], in_=bass.AP(
            tensor=t[wname].ap().tensor, offset=0,
            ap=[[KT * C, 128], [1, KT * C]]))
        return wh

    xg = t['xg'].ap()
    out_ap = t['out2d'].ap()
    rt_ap = t['rtab'].ap()

    for b in range(c.B_SH):
        # ---- A1: xT load via xbar DMA transpose (channel-major) ----
        xT = big.tile([128, KT, L], BF16, tag="S1")
        for kt in range(KT):
            nc.sync.dma_start_transpose(
                out=xT[:, kt, :],
                in_=xg[b * L:(b + 1) * L, 128 * kt:128 * (kt + 1)])

        if b == 0:
            # rtab is first needed in the v-pass; loading it here keeps the
            # 4MB transfer out of the DMA queue ahead of the gating x loads
            nc.sync.dma_start(out=rtab_sb[:], in_=t['rtab'].ap())

        # ---- A2: x_seq direct load from host-blocked layout [p, b, lt, c]
        # (one contiguous 32KB run per partition per batch) ----
        x_seq = big.tile([128, NS, C], BF16, tag="S2")
        nc.sync.dma_start(out=x_seq[:], in_=bass.AP(
            tensor=t['xgb'].ap().tensor, offset=b * NS * C,
            ap=[[c.B_SH * NS * C, 128], [1, NS * C]]))

        # ---- A3: q-pass -> gq (channel-major, padded by 2) ----
        gq = big.tile([128, KT, L + 4], BF16, tag="S3")
        nc.gpsimd.memset(gq[:, :, 0:4], 0.0)
        wq_h = load_weight('wq')
        for hi in range(NO):
            for mm_ in range(MH):
                m = hi * MH + mm_
                for n in range(NB):
                    ps = psmm.tile([128, 512], F32, tag="mm", space="PSUM")
                    for kt in range(KT):
                        nc.tensor.matmul(ps[:], lhsT=wq_h[:, hi, kt, 128 * mm_:128 * (mm_ + 1)],
                                         rhs=xT[:, kt, 512 * n:512 * (n + 1)],
                                         start=(kt == 0), stop=(kt == KT - 1))
                    nc.scalar.activation(out=gq[:, m, 4 + 512 * n:4 + 512 * (n + 1)], in_=ps[:],
                                         func=AF.Identity, bias=bq_col_sb[:, m:m + 1], scale=1.0)

        # ---- A4+A5: per-group conv offsets -> tent S -> sampled xs ----
        # qT transposes are interleaved into the (g, lt) loop to give the PE
        # filler work while the tent chains run on DVE/ACT.
        xs = big.tile([128, KT, L], BF16, tag="S4")
        for g in range(G):
            # fused offset conv: off_raw(l') = sum_{cin,t} Wc[cin,t] q_g(l'+t-4)
            # (w2 folded through conv1 on host; h never materialized)
            offp = psx.tile([128, 128], F32, tag="sc64", space="PSUM")
            for s_ in range(NS):
                for kt2 in range(SLG):
                    for tt_ in range(K):
                        nc.tensor.matmul(
                            offp[:, s_:s_ + 1],
                            lhsT=gq[:, g * SLG + kt2, 128 * s_ + tt_:128 * s_ + tt_ + 128],
                            rhs=wc_sb[:, kt2, tt_:tt_ + 1],
                            start=(kt2 == 0 and tt_ == 0),
                            stop=(kt2 == SLG - 1 and tt_ == K - 1))
            # reference zero-pads h AFTER bias: l' in {0,1} must see off_raw = b2
            # (offc = -w2.b1 so that tanh bias (b2 + w2.b1) cancels back to b2)
            nc.vector.tensor_copy(out=offp[0:2, 0:1], in_=offc_sb[0:2, :])

            tanh_t = sm.tile([128, NS], F32, tag="tanh")
            nc.scalar.activation(out=tanh_t[:], in_=offp[:, 0:NS], func=AF.Tanh,
                                 bias=boff2_sb[:, 0:1], scale=1.0)
            psm = sm.tile([128, NS], F32, tag="psm")
            nc.vector.tensor_scalar(out=psm[:], in0=tanh_t[:], scalar1=float(K) * c.sn,
                                    scalar2=None, op0=OP.mult)
            nc.vector.tensor_tensor(out=psm[:], in0=psm[:], in1=posg_sb[:], op=OP.add)

            for lt in range(NS):
                # t = l - pos(l'); window l in [128lt-64, 128lt+144) since
                # pos - l' in [-8.5, 4.5]. tent = relu(1 - |t|).
                nsc = sm.tile([128, 1], F32, tag="scl")
                nc.vector.tensor_scalar(out=nsc[:], in0=psm[:, lt:lt + 1],
                                        scalar1=-1.0, scalar2=128.0 * lt - 64.0,
                                        op0=OP.mult, op1=OP.add)
                tdif = smb2.tile([128, 208], BF16, tag="tdif")
                nc.vector.tensor_scalar(out=tdif[:], in0=iota_sb[:], scalar1=nsc[:, 0:1],
                                        scalar2=None, op0=OP.add)
                ab = smb.tile([128, 208], BF16, tag="r1t")
                nc.scalar.activation(out=ab[:], in_=tdif[:], func=AF.Abs)
                st = sm.tile([128, 208], BF16, tag="st")
                nc.scalar.activation(out=st[:], in_=ab[:], func=AF.Relu,
                                     scale=-1.0, bias=1.0)
                # middle chunk [64:192] full 128; side A [0:64] -> base-64 rows;
                # side C [192:208] -> first 16 rows of block lt+1
                S_mid = sm.tile([128, 128], BF16, tag="Smid")
                pt = pstb.tile([128, 128], BF16, tag="trb", space="PSUM")
                nc.tensor.transpose(pt[:], st[:, 64:192], identb[:])
                nc.vector.tensor_copy(out=S_mid[:], in_=pt[:])
                S_lo = sm.tile([128, 128], BF16, tag="Slo")
                S_hi = sm.tile([16, 128], BF16, tag="Shi")
                if lt > 0:
                    ptA = pstb.tile([128, 128], BF16, tag="trb", space="PSUM")
                    nc.tensor.transpose(ptA[64:128, :], st[:, 0:64], identb[:],
                                        tile_position=(0, 64))
                    nc.vector.tensor_copy(out=S_lo[64:128, :], in_=ptA[64:128, :])
                if lt < NS - 1:
                    ptC = pstb.tile([128, 128], BF16, tag="trb", space="PSUM")
                    nc.tensor.transpose(ptC[0:16, :], st[:, 192:208], identb[:])
                    nc.vector.tensor_copy(out=S_hi[:], in_=ptC[0:16, :])
                for ct in range(SLG):
                    kk = SLG * g + ct
                    psS = ps1f.tile([128, 128], F32, tag="p128f", space="PSUM")
                    nc.tensor.matmul(psS[:],
                                     lhsT=x_seq[:, lt, 128 * kk:128 * (kk + 1)],
                                     rhs=S_mid[:],
                                     start=True, stop=(lt == 0 and lt == NS - 1))
                    if lt > 0:
                        nc.tensor.matmul(psS[:],
                                         lhsT=x_seq[64:128, lt - 1, 128 * kk:128 * (kk + 1)],
                                         rhs=S_lo[64:128, :],
                                         start=False, stop=(lt == NS - 1))
                    if lt < NS - 1:
                        nc.tensor.matmul(psS[:],
                                         lhsT=x_seq[0:16, lt + 1, 128 * kk:128 * (kk + 1)],
                                         rhs=S_hi[:],
                                         start=False, stop=True)
                    nc.scalar.activation(out=xs[:, kk, 128 * lt:128 * (lt + 1)], in_=psS[:],
                                         func=AF.Identity)

        # ---- A6: qT via PE transposes of gq ----
        qT = big.tile([128, NS, C], BF16, tag="S2")
        for m in range(KT):
            for lt in range(NS):
                pt = pstb.tile([128, 128], BF16, tag="trb", space="PSUM")
                nc.tensor.transpose(pt[:], gq[:, m, 4 + 128 * lt:4 + 128 * (lt + 1)], identb[:])
                nc.vector.tensor_copy(out=qT[:, lt, 128 * m:128 * (m + 1)], in_=pt[:])

        # ---- A7: kT-pass ----
        kT = big.tile([128, NS, C], BF16, tag="S1")
        wk_h = load_weight('wk')
        for hi in range(NO):
            for lt in range(NS):
                ps = psmm.tile([128, 512], F32, tag="mm", space="PSUM")
                for kt in range(KT):
                    nc.tensor.matmul(ps[:], lhsT=xs[:, kt, 128 * lt:128 * (lt + 1)],
                                     rhs=wk_h[:, hi, kt, :],
                                     start=(kt == 0), stop=(kt == KT - 1))
                nc.vector.tensor_copy(out=kT[:, lt, 512 * hi:512 * (hi + 1)], in_=ps[:])

        # ---- A8: scores + softmax + transposed block-diag attn ----
        attnTs = []
        for pr in range(c.P2):
            # one [128,128] matmul per lt covers both heads of the pair; only
            # the two diagonal 64x64 quadrants are meaningful.
            ps_sc = psx.tile([128, 128], F32, tag="sc64", space="PSUM")
            for lt in range(NS):
                nc.tensor.matmul(ps_sc[:],
                                 lhsT=qT[:, lt, 128 * pr:128 * (pr + 1)],
                                 rhs=kT[:, lt, 128 * pr:128 * (pr + 1)],
                                 start=(lt == 0), stop=(lt == NS - 1))
            rmax = sm.tile([128, 1], F32, tag="rmax")
            nc.vector.reduce_max(out=rmax[0:64, :], in_=ps_sc[0:64, 0:64],
                                 axis=mybir.AxisListType.X)
            nc.vector.reduce_max(out=rmax[64:128, :], in_=ps_sc[64:128, 64:128],
                                 axis=mybir.AxisListType.X)
            nb_ = sm.tile([128, 1], F32, tag="nb")
            nc.vector.tensor_scalar(out=nb_[:], in0=rmax[:], scalar1=-scale, scalar2=None, op0=OP.mult)
            expt = sm.tile([128, 64], F32, tag="expt")
            nc.scalar.activation(out=expt[0:64, :], in_=ps_sc[0:64, 0:64], func=AF.Exp,
                                 bias=nb_[0:64, :], scale=scale)
            nc.scalar.activation(out=expt[64:128, :], in_=ps_sc[64:128, 64:128], func=AF.Exp,
                                 bias=nb_[64:128, :], scale=scale)
            rsum = sm.tile([128, 1], F32, tag="rsum")
            nc.vector.reduce_sum(out=rsum[:], in_=expt[:], axis=mybir.AxisListType.X)
            rinv = sm.tile([128, 1], F32, tag="rinv")
            nc.vector.reciprocal(out=rinv[:], in_=rsum[:])
            ablk = smb.tile([128, 128], BF16, tag="ablk")
            nc.gpsimd.memset(ablk[:], 0.0)
            nc.vector.tensor_scalar(out=ablk[0:64, 0:64], in0=expt[0:64, :],
                                    scalar1=rinv[0:64, :], scalar2=None, op0=OP.mult)
            nc.vector.tensor_scalar(out=ablk[64:128, 64:128], in0=expt[64:128, :],
                                    scalar1=rinv[64:128, :], scalar2=None, op0=OP.mult)
            trp = pstb.tile([128, 128], BF16, tag="trb", space="PSUM")
            nc.tensor.transpose(trp[:], ablk[:], identb[:])
            aT = ap_.tile([128, 128], BF16, tag=f"aT{pr}")
            nc.vector.tensor_copy(out=aT[:], in_=trp[:])
            attnTs.append(aT)

        # ---- A9: v-pass (+ rtab bias) ----
        v = big.tile([128, KT, L], BF16, tag="S3")
        wv_h = load_weight('wv')
        for hi in range(NO):
            for mm_ in range(MH):
                m = hi * MH + mm_
                for n in range(NB):
                    ps = psmm.tile([128, 512], F32, tag="mm", space="PSUM")
                    for kt in range(KT):
                        nc.tensor.matmul(ps[:], lhsT=wv_h[:, hi, kt, 128 * mm_:128 * (mm_ + 1)],
                                         rhs=xs[:, kt, 512 * n:512 * (n + 1)],
                                         start=(kt == 0), stop=(kt == KT - 1))
                    nc.vector.tensor_tensor(out=v[:, m, 512 * n:512 * (n + 1)],
                                            in0=ps[:], in1=rtab_sb[:, m * L + 512 * n:m * L + 512 * (n + 1)],
                                            op=OP.add)

        # ---- A10: attn @ v -> ao (channel-major) ----
        ao = big.tile([128, KT, L], BF16, tag="S4")
        for pr in range(c.P2):
            for n in range(NB):
                ps = psmm.tile([128, 512], F32, tag="mm", space="PSUM")
                nc.tensor.matmul(ps[:], lhsT=attnTs[pr][:],
                                 rhs=v[:, pr, 512 * n:512 * (n + 1)],
                                 start=True, stop=True)
                nc.vector.tensor_copy(out=ao[:, pr, 512 * n:512 * (n + 1)], in_=ps[:])

        # ---- A11: out-pass, seq-major, contiguous row stores ----
        wo_h = load_weight('wo')
        for ltp in range(NS // 2):
            yt = yp.tile([128, 2, C], BF16, tag="yt")
            for z in range(2):
                lt = 2 * ltp + z
                for hi in range(NO):
                    ps = psmm.tile([128, 512], F32, tag="mm", space="PSUM")
                    for kt in range(KT):
                        nc.tensor.matmul(ps[:], lhsT=ao[:, kt, 128 * lt:128 * (lt + 1)],
                                         rhs=wo_h[:, hi, kt, :],
                                         start=(kt == 0), stop=(kt == KT - 1))
                    nc.vector.tensor_copy(out=yt[:, z, 512 * hi:512 * (hi + 1)], in_=ps[:])
            # custom out2d layout [p, b, ltp, z, c]: one contiguous 4KB run
            # per partition per store; host untangles the layout
            nc.sync.dma_start(out=bass.AP(
                tensor=out_ap.tensor,
                offset=(b * (NS // 2) + ltp) * 2 * C,
                ap=[[c.B_SH * NS * C, 128], [1, 2 * C]]), in_=yt[:])


def make_nc(cfg):
    nc = bacc.Bacc("TRN2", target_bir_lowering=False, debug=False)
    t = declare(nc, cfg)
    from contextlib import ExitStack
    with tile.TileContext(nc) as tc:
        with ExitStack() as ctx:
            build(tc, t, cfg, ctx)
    nc.compile()
    return nc


def _blk(wT, KT, C):
    # wT: [C_in, C_out] -> [128, hi, kt, 512] flattened to [128, KT*C]
    NO = C // 512
    return np.ascontiguousarray(
        wT.reshape(KT, 128, NO, 512).transpose(1, 2, 0, 3).reshape(128, KT * C))


def host_prep_shared(inputs, cfg):
    c = cfg
    sh = {
        'wq': _blk(inputs['Wq'].T.astype(BF), c.KT, c.C),
        'wk': _blk(inputs['Wk'].T.astype(BF), c.KT, c.C),
        'wv': _blk(inputs['Wv'].T.astype(BF), c.KT, c.C),
        'wo': _blk(inputs['Wout'].T.astype(BF), c.KT, c.C),
        'wc': np.ascontiguousarray(
            np.einsum('c,cit->it', np.asarray(inputs['Woff2'][0, :, 0], np.float64),
                      np.asarray(inputs['Woff1'], np.float64)).astype(BF)
            .reshape(c.SLG, 128, c.K).transpose(1, 0, 2).reshape(128, c.SLG * c.K)),
        'offc': np.full((128, 1),
                        -float(np.dot(np.asarray(inputs['Woff2'][0, :, 0], np.float64),
                                      np.asarray(inputs['boff1'], np.float64))),
                        np.float32),
        'bq_col': np.ascontiguousarray(
            inputs['bq'].astype(np.float32).reshape(c.KT, 128).T),
        'bk_row': inputs['bk'][None, :].astype(BF),
        'bout_row': inputs['bout'][None, :].astype(BF),
        'boff2c': (inputs['boff2'][:, None]
                   + np.dot(np.asarray(inputs['Woff2'][0, :, 0], np.float64),
                            np.asarray(inputs['boff1'], np.float64))).astype(np.float32),
        'posg': ((np.arange(128)[:, None] + 128 * np.arange(c.NS)[None, :]) * c.sn
                 - 0.5).astype(np.float32),
        'iota': np.arange(208, dtype=np.float32)[None, :].astype(BF),
        'rtab': np.ascontiguousarray(
            (inputs['bv'][:, None] + inputs['rpb_table'][0]).astype(BF)
            .reshape(c.KT, 128, c.L).transpose(1, 0, 2).reshape(128, c.KT * c.L)),
    }
    return sh


def host_prep_core(x_shard, cfg):
    c = cfg
    xbf = x_shard.reshape(c.B_SH * c.L, c.C).astype(BF)
    xgb = (xbf.reshape(c.B_SH, c.NS, 128, c.C).transpose(2, 0, 1, 3)
           .reshape(128, c.B_SH * c.NS * c.C))
    return {'xg': np.ascontiguousarray(xbf), 'xgb': np.ascontiguousarray(xgb)}


# ----------------------------------------------------------------------------
# Public entry point
# ----------------------------------------------------------------------------
_N_CORES = 8
_B, _L, _C, _H, _G, _K = 16, 2048, 1024, 16, 4, 5
_CACHE = {}


def _get_nc(cfg):
    if 'nc' not in _CACHE:
        _CACHE['nc'] = make_nc(cfg)
    return _CACHE['nc']


def kernel(**inputs):
    inputs = {k: np.asarray(v) for k, v in inputs.items()}
    cfg = Cfg(B_SH=_B // _N_CORES, L=_L, C=_C, H=_H, G=_G, K=_K)
    nc = _get_nc(cfg)
    sh = host_prep_shared(inputs, cfg)
    in_maps = [
        {**sh, **host_prep_core(inputs['x'][c * cfg.B_SH:(c + 1) * cfg.B_SH], cfg)}
        for c in range(_N_CORES)
    ]
    from concourse.bass_utils import run_bass_kernel_spmd
    res = run_bass_kernel_spmd(nc, in_maps, core_ids=list(range(_N_CORES)))
    outs = []
    for cc in range(_N_CORES):
        arr = res.results[cc]["out2d"].reshape(128, cfg.B_SH, _L // 256, 2, _C)
        outs.append(arr.transpose(1, 2, 3, 0, 4).reshape(cfg.B_SH, _L, _C))
    return np.concatenate(outs, axis=0).astype(np.float32)
